# revision 1
# baseline (speedup 1.0000x reference)
"""Block-diagonal ZF equalizer (nn_BDEqualizer) as a Trainium2 Bass kernel.

Math: for every resource element (b, s, f) and UE u, solve the 8x8 complex
system H_u x_u = y_u where H_u[i, j] = h[b, 0, 8u+i, u, j, s, f] and
y_u[i] = y[b, 0, 8u+i, s, f].  Output x as [B, 1, 32, S, F, 2] (re/im last).

Strategy (data-parallel over the fft axis, per the sharding hint):
  - 8 cores, each owns a contiguous 128-subcarrier slice of F=1024.
  - Host pre-extracts the block-diagonal channel blocks (pure indexing) and
    ships per-core shards hd[B, U, 8, 8, S, 128] / yd[B, U, 8, S, 128].
  - On-chip layout: subcarriers on the 128 SBUF partitions, the other RE
    axes (u, b-pair, s) = 112 along the free dim.  Each of the 9 augmented
    matrix columns (8 of H + rhs) is a "plane" of 8 rows; every Gaussian
    elimination step is a full-width [128, n*112] elementwise op on the
    Vector engine, with per-RE pivot reciprocals.  Unpivoted LU + Jordan
    back-substitution, complex arithmetic as separate re/im tiles.
  - TensorE transposes move between the DMA-friendly [(u,b,s), f] staging
    layout and the compute layout [f, (u,b,s)]; ScalarE drains PSUM.
  - Two chunks (b in {0,1} then {2,3}) double-buffer load against compute.
  - Elimination updates run on groups of up to 4 planes per instruction
    (the plane index rides a third AP dim: [128, w, n, 112] broadcast
    views), cutting VectorE instruction-issue overhead ~2x; pivot squares
    go to ScalarE; each solution row is stored (TensorE transpose + DMA)
    as soon as its back-substitution step finishes, hiding the store under
    the remaining VectorE back pass.  The kernel is VectorE-bound (fp32
    tensor_tensor is 1 elem/lane/cycle and the ~38M-element-op/core solve
    has no matmul structure for TensorE, while GPSIMD is locked out of the
    shared SBUF port during 2-port DVE ops).

Measured: cost-model (TimelineSim) per-core time ~496 us (~94% VectorE
busy, vs a ~420 us pure element-streaming floor); output vs the fp32 jax
reference: rel-L2 2.9e-4, abs-max 0.77 on a +-1203 output (6.4e-4
scale-relative; unpivoted GE tail on the worst-conditioned REs).
"""

import os

import numpy as np

import concourse.bacc as bacc
import concourse.mybir as mybir
from concourse.bass_utils import run_bass_kernel_spmd
from concourse.masks import make_identity
from concourse.tile import TileContext

B, NRX, NR, U, A, S, F = 4, 1, 32, 4, 8, 14, 1024
NCORES = 8
FS = F // NCORES        # 128 subcarriers per core
NB = 2                  # batch entries per chunk
NCH = B // NB           # chunks per core
M = U * NB * S          # 112 RE columns per chunk (u, b, s)
NP = 9                  # augmented planes: 8 matrix columns + rhs
F32 = mybir.dt.float32
AL = mybir.AluOpType

LAST_RESULTS = None     # BassKernelResults of the most recent run (for test.py)


def _off(j, i):
    """Free-dim offset of (plane j, row i) inside an H supertile."""
    return (j * A + i) * M


def _build():
    nc = bacc.Bacc(trn_type="TRN2")

    # Host-prepped layouts, chosen so every per-(chunk, i) DMA slice is
    # stride-collapsible: hd[i, u, b, s, j, f], yd[i, u, b, s, f],
    # out[i, u, b, s, f, c].  (i = matrix row, j = matrix column.)
    hdre = nc.dram_tensor("hd_re", [A, U, B, S, A, FS], F32, kind="ExternalInput")
    hdim = nc.dram_tensor("hd_im", [A, U, B, S, A, FS], F32, kind="ExternalInput")
    ydre = nc.dram_tensor("yd_re", [A, U, B, S, FS], F32, kind="ExternalInput")
    ydim = nc.dram_tensor("yd_im", [A, U, B, S, FS], F32, kind="ExternalInput")
    out = nc.dram_tensor("out", [A, U, B, S, FS, 2], F32, kind="ExternalOutput")

    with TileContext(nc) as tc:
        with (
            tc.tile_pool(name="consts", bufs=1) as consts,
            tc.tile_pool(name="supers", bufs=2) as supers,
            tc.tile_pool(name="work", bufs=1) as work,
            tc.tile_pool(name="stg", bufs=3) as stg,
            tc.tile_pool(name="stgo", bufs=3) as stgo,
            tc.tile_pool(name="psin", bufs=3, space="PSUM") as psin,
            tc.tile_pool(name="psy", bufs=2, space="PSUM") as psy_pool,
            tc.tile_pool(name="pso", bufs=2, space="PSUM") as pso_pool,
        ):
            ident = consts.tile([128, 128], F32)
            make_identity(nc, ident)

            for ci in range(NCH):
                b0 = ci * NB
                HRe = supers.tile([128, (NP + 1) * A * M], F32, tag="HRe")
                HIm = supers.tile([128, (NP + 1) * A * M], F32, tag="HIm")
                hsup = (HRe, HIm)

                def row(T, j, i):
                    return T[:, _off(j, i) : _off(j, i) + M]

                def rows3(T, j, i0, n):
                    base = _off(j, i0)
                    return T[:, base : base + n * M].rearrange(
                        "p (r c) -> p r c", r=n
                    )

                def bc(ap, n):
                    return ap[:, None, :].broadcast_to([128, n, M])

                # ---------------- load h ----------------
                for comp in range(2):
                    hsrc = (hdre, hdim)[comp]
                    for i in range(A):
                        stage = stg.tile([M, A * FS], F32, tag="stage")
                        src = hsrc[i, :, b0 : b0 + NB]
                        nc.sync.dma_start(stage, src)
                        for jg in range(2):
                            ps = psin.tile([128, 4 * M], F32, tag="psin")
                            for q in range(4):
                                j = jg * 4 + q
                                nc.tensor.transpose(
                                    ps[:, q * M : (q + 1) * M],
                                    stage[:, j * FS : (j + 1) * FS],
                                    ident[:M, :M],
                                )
                            base = _off(jg * 4, i)
                            dst = hsup[comp][:, base : base + 4 * A * M].rearrange(
                                "p (q c) -> p q c", q=4
                            )[:, :, :M]
                            src3 = ps.rearrange("p (q c) -> p q c", q=4)
                            nc.scalar.copy(dst, src3)

                # ---------------- load y ----------------
                for comp in range(2):
                    ysrc = (ydre, ydim)[comp]
                    for i in range(A):
                        sy = stg.tile([M, FS], F32, tag="stagey")
                        nc.sync.dma_start(sy, ysrc[i, :, b0 : b0 + NB])
                        py = psy_pool.tile([128, M], F32, tag="psy")
                        nc.tensor.transpose(py, sy, ident[:M, :M])
                        nc.scalar.copy(row(hsup[comp], 8, i), py)

                # ---------------- solve ----------------
                # INV holds the pivot reciprocals: ir block [0:A*M], ii block
                # [A*M:2*A*M], plus A*M padding so the (ir_k, ii_k) stride-
                # A*M pair view can be built by slice+rearrange for every k.
                INV = work.tile([128, 3 * A * M], F32, tag="INV")
                FRe = work.tile([128, (A - 1) * M], F32, tag="FRe")
                FIm = work.tile([128, (A - 1) * M], F32, tag="FIm")
                # PAs is the single wide product scratch for the width-4
                # elimination groups (DVE is in-order, so product->accumulate
                # can reuse one buffer); PBs only needs the factor-prep pair.
                PAs = work.tile([128, 4 * (A - 1) * M], F32, tag="PAs")
                PBs = work.tile([128, 4 * (A - 1) * M], F32, tag="PBs")
                PCs = work.tile([128, (A - 1) * M], F32, tag="PCs")
                PDs = work.tile([128, (A - 1) * M], F32, tag="PDs")
                TD = work.tile([128, M], F32, tag="TD")
                TU = work.tile([128, M], F32, tag="TU")
                TR = work.tile([128, M], F32, tag="TR")

                def sc3(T, n):
                    return T[:, : n * M].rearrange("p (r c) -> p r c", r=n)

                def sc4(T, n):
                    # [128, 2, n, M] j-major view of scratch
                    return T[:, : 2 * n * M].rearrange(
                        "p (j r c) -> p j r c", j=2, r=n
                    )

                def sc_half(T, h, n):
                    return T[:, h * n * M : (h + 1) * n * M]

                def inv_pair(k, n=None):
                    # (ir_k, ii_k) as [128, 2, M]; broadcast over n rows if set
                    v = INV[:, k * M : k * M + 2 * A * M].rearrange(
                        "p (j c) -> p j c", j=2
                    )[:, :, :M]
                    if n is None:
                        return v
                    return v[:, :, None, :].broadcast_to([128, 2, n, M])

                # forward elimination
                for k in range(A):
                    a = row(HRe, k, k)
                    b_ = row(HIm, k, k)
                    nc.scalar.square(TD, a)
                    nc.scalar.square(TU, b_)
                    nc.vector.tensor_add(TD, TD, TU)
                    nc.vector.reciprocal(TR, TD)
                    irk = INV[:, k * M : (k + 1) * M]
                    iik = INV[:, (A + k) * M : (A + k + 1) * M]
                    nc.vector.tensor_mul(irk, a, TR)
                    nc.vector.tensor_mul(iik, b_, TR)
                    n = A - 1 - k
                    if n == 0:
                        continue
                    # factors F = -H[i,k] * inv(p), via paired products:
                    #   P1 = (a*ir || a*ii),  P2 = (b*ir || b*ii)
                    car = rows3(HRe, k, k + 1, n)
                    cai = rows3(HIm, k, k + 1, n)
                    car4 = car[:, None, :, :].broadcast_to([128, 2, n, M])
                    cai4 = cai[:, None, :, :].broadcast_to([128, 2, n, M])
                    nc.vector.tensor_mul(sc4(PAs, n), car4, inv_pair(k, n))
                    nc.vector.tensor_mul(sc4(PBs, n), cai4, inv_pair(k, n))
                    fre = FRe[:, : n * M]
                    fim = FIm[:, : n * M]
                    # fre = -(a*ir + b*ii), fim = a*ii - b*ir
                    nc.vector.scalar_tensor_tensor(
                        fre, sc_half(PAs, 0, n), -1.0, sc_half(PBs, 1, n),
                        AL.mult, AL.subtract,
                    )
                    nc.vector.tensor_sub(
                        fim, sc_half(PAs, 1, n), sc_half(PBs, 0, n)
                    )
                    # eliminate column k from planes k+1..7 and y, in groups
                    # of up to 4 planes per instruction: the plane index is a
                    # third AP dim (stride A*M), so one [128, w, n, M] op
                    # covers w planes.  Products cycle through the single
                    # scratch PAs; the in-order DVE serializes them anyway.
                    js = list(range(k + 1, NP))
                    while js:
                        w = min(4, len(js))
                        j0 = js[0]
                        js = js[w:]

                        def wrows(T):
                            base = _off(j0, k + 1)
                            return T[:, base : base + w * A * M].rearrange(
                                "p (w c) -> p w c", w=w
                            )[:, :, : n * M]

                        def wrow_b(T):
                            base = _off(j0, k)
                            v = T[:, base : base + w * A * M].rearrange(
                                "p (w c) -> p w c", w=w
                            )[:, :, :M]
                            return v[:, :, None, :].broadcast_to(
                                [128, w, n, M]
                            )

                        def fw(Ft):
                            v = Ft[:, : n * M].rearrange(
                                "p (r c) -> p r c", r=n
                            )
                            return v[:, None, :, :].broadcast_to(
                                [128, w, n, M]
                            )

                        hr, hi = wrows(HRe), wrows(HIm)
                        Br, Bi = wrow_b(HRe), wrow_b(HIm)
                        frew, fimw = fw(FRe), fw(FIm)
                        SA4 = PAs[:, : w * n * M].rearrange(
                            "p (w r c) -> p w r c", w=w, r=n
                        )
                        SA3 = PAs[:, : w * n * M].rearrange(
                            "p (w c) -> p w c", w=w
                        )
                        SB4 = PBs[:, : w * n * M].rearrange(
                            "p (w r c) -> p w r c", w=w, r=n
                        )
                        SB3 = PBs[:, : w * n * M].rearrange(
                            "p (w c) -> p w c", w=w
                        )
                        # H[i,j] += F*B (complex); products regrouped by
                        # factor so consecutive VectorE ops never share a
                        # RAW destination (longer dep gaps -> less ack stall)
                        nc.vector.tensor_mul(SA4, frew, Br)
                        nc.vector.tensor_mul(SB4, frew, Bi)
                        nc.vector.tensor_add(hr, hr, SA3)
                        nc.vector.tensor_add(hi, hi, SB3)
                        nc.vector.tensor_mul(SA4, fimw, Bi)
                        nc.vector.tensor_mul(SB4, fimw, Br)
                        nc.vector.tensor_sub(hr, hr, SA3)
                        nc.vector.tensor_add(hi, hi, SB3)

                # back substitution (Jordan): x_k = y_k*invp, then clear col k
                for k in range(A - 1, -1, -1):
                    yr = row(HRe, 8, k)
                    yi = row(HIm, 8, k)
                    # P1 = (yr*ir || yr*ii), P2 = (yi*ir || yi*ii)
                    p1 = PAs[:, : 2 * M].rearrange("p (j c) -> p j c", j=2)
                    p2 = PBs[:, : 2 * M].rearrange("p (j c) -> p j c", j=2)
                    yr2 = yr[:, None, :].broadcast_to([128, 2, M])
                    yi2 = yi[:, None, :].broadcast_to([128, 2, M])
                    nc.vector.tensor_mul(p1, yr2, inv_pair(k))
                    nc.vector.tensor_mul(p2, yi2, inv_pair(k))
                    # x = y * conj(p)/|p|^2: xr = yr*ir + yi*ii, xi = yi*ir - yr*ii
                    nc.vector.tensor_add(yr, PAs[:, :M], PBs[:, M : 2 * M])
                    nc.vector.tensor_sub(yi, PBs[:, :M], PAs[:, M : 2 * M])
                    # x_k is final now -- store it while the rest of the back
                    # pass still runs on VectorE.
                    so = stgo.tile([M, 2 * FS], F32, tag="so")
                    so3 = so.rearrange("p (f c) -> p f c", c=2)
                    for comp in range(2):
                        po = pso_pool.tile([M, FS], F32, tag="pso")
                        nc.tensor.transpose(
                            po, row(hsup[comp], 8, k), ident[:128, :128]
                        )
                        nc.scalar.copy(so3[:, :, comp], po)
                    dst = out[k, :, b0 : b0 + NB]
                    nc.sync.dma_start(dst, so)
                    if k == 0:
                        continue
                    cr = rows3(HRe, k, 0, k)
                    ci_ = rows3(HIm, k, 0, k)
                    xrB = bc(yr, k)
                    xiB = bc(yi, k)
                    qa, qb, qc, qd = (sc3(t, k) for t in (PAs, PBs, PCs, PDs))
                    nc.vector.tensor_mul(qa, cr, xrB)
                    nc.vector.tensor_mul(qb, ci_, xiB)
                    nc.vector.tensor_mul(qc, cr, xiB)
                    nc.vector.tensor_mul(qd, ci_, xrB)
                    ytr = rows3(HRe, 8, 0, k)
                    yti = rows3(HIm, 8, 0, k)
                    # y_i -= H[i,k] * x_k
                    nc.vector.tensor_sub(ytr, ytr, qa)
                    nc.vector.tensor_add(ytr, ytr, qb)
                    nc.vector.tensor_sub(yti, yti, qc)
                    nc.vector.tensor_sub(yti, yti, qd)


    nc.finalize()
    return nc


_NC_CACHE = None


def _get_nc():
    global _NC_CACHE
    if _NC_CACHE is None:
        _NC_CACHE = _build()
    return _NC_CACHE


def _prep_core(y_re, y_im, h_re, h_im, c):
    """Host-side shard prep for core c: f-slice + block-diagonal extraction."""
    fsl = slice(c * FS, (c + 1) * FS)
    ue = np.arange(U)
    maps = {}
    for name, h in (("hd_re", h_re), ("hd_im", h_im)):
        h6 = h[:, 0, :, :, :, :, fsl].reshape(B, U, A, U, A, S, FS)
        hd = h6[:, ue, :, ue]              # [u, b, i, j, s, f]
        maps[name] = np.ascontiguousarray(
            hd.transpose(2, 0, 1, 4, 3, 5), dtype=np.float32
        )                                   # [i, u, b, s, j, f]
    for name, y in (("yd_re", y_re), ("yd_im", y_im)):
        y5 = y[:, 0, :, :, fsl].reshape(B, U, A, S, FS)   # [b, u, i, s, f]
        maps[name] = np.ascontiguousarray(
            y5.transpose(2, 1, 0, 3, 4), dtype=np.float32
        )                                   # [i, u, b, s, f]
    return maps


def kernel(y_re, y_im, h_re, h_im, **_ignored):
    global LAST_RESULTS
    y_re = np.asarray(y_re, dtype=np.float32)
    y_im = np.asarray(y_im, dtype=np.float32)
    h_re = np.asarray(h_re, dtype=np.float32)
    h_im = np.asarray(h_im, dtype=np.float32)

    nc = _get_nc()
    in_maps = [_prep_core(y_re, y_im, h_re, h_im, c) for c in range(NCORES)]
    trace = bool(int(os.environ.get("BD_TRACE", "0")))
    res = run_bass_kernel_spmd(
        nc, in_maps, core_ids=list(range(NCORES)), trace=trace
    )
    LAST_RESULTS = res
    outs = []
    for r in res.results:
        o = r["out"]                              # [i, u, b, s, f, c]
        o = o.transpose(2, 1, 0, 3, 4, 5)         # [b, u, i, s, f, c]
        outs.append(o.reshape(B, NR, S, FS, 2))
    full = np.concatenate(outs, axis=3)           # [B, NR, S, F, 2]
    return np.ascontiguousarray(full[:, None])    # [B, 1, NR, S, F, 2]



# revision 16
# speedup vs baseline: 1.4074x; 1.4074x over previous
"""Block-diagonal ZF equalizer (nn_BDEqualizer) as a Trainium2 Bass kernel.

Math: for every resource element (b, s, f) and UE u, solve the 8x8 complex
system H_u x_u = y_u where H_u[i, j] = h[b, 0, 8u+i, u, j, s, f] and
y_u[i] = y[b, 0, 8u+i, s, f].  Output x as [B, 1, 32, S, F, 2] (re/im last).

Strategy (data-parallel over the fft axis, per the sharding hint):
  - 8 cores, each owns a contiguous 128-subcarrier slice of F=1024.
  - The host pre-extracts the block-diagonal channel blocks AND pre-
    transposes them into the exact on-chip compute layout (subcarriers on
    the 128 SBUF partitions, the (j-plane, i-row, u/b/s) RE axes along the
    free dim), so loads and stores are plain partition-major DMAs: no
    on-chip transposes, no staging, no PSUM drains on the load path.
  - Unpivoted complex Gaussian elimination on the 9-plane augmented
    supertile, fp32 throughout, software-pipelined so consecutive steps
    overlap: plane k+1 is updated first (on DVE, the critical path), then
    step k+1's pivot/factors are computed while the remaining planes of
    step k stream through the worker lanes.
  - Three elementwise lanes, balanced at build time by a greedy cost
    tracker: DVE (1.04 ns/elem), Pool (1.98 ns/elem), and the Tensor
    engine as an adder lane - identity-weight fp32 matmuls accumulate
    (H + P1 +- P2) in PSUM (weight loads are free) with ScalarE draining
    the result back to SBUF.  Complex products are emitted as interleaved
    pairs ((fr||fi) * bcast(b)), one instruction covering two planes.
"""

import os

import numpy as np

import concourse.bacc as bacc
import concourse.mybir as mybir
from concourse.bass_utils import run_bass_kernel_spmd
from concourse.masks import make_identity
from concourse.tile import TileContext

B, NRX, NR, U, A, S, F = 4, 1, 32, 4, 8, 14, 1024
NCORES = 8
FS = F // NCORES        # 128 subcarriers per core
NB = 2                  # batch entries per chunk
NCH = B // NB           # chunks per core
M = U * NB * S          # 112 RE columns per chunk (u, b, s)
NP = 9                  # augmented planes: 8 matrix columns + rhs
F32 = mybir.dt.float32
AL = mybir.AluOpType

LAST_RESULTS = None     # BassKernelResults of the most recent run (for test.py)


def _off(j, i):
    """Free-dim offset of (plane j, row i) inside an H supertile."""
    return (j * A + i) * M


class _Balancer:
    """Greedy build-time engine load balancer."""

    RATE = {"V": 1.042, "P": 1.984}
    FIX = {"V": 62.0, "P": 8.0}
    PE_RATE = 3 * 1.8           # 3 fp32 matmul rows per pair elem
    PE_FIX = 30.0
    ACT_RATE = 0.833
    ACT_FIX = 150.0

    def __init__(self):
        self.busy = {"V": 0.0, "P": 0.0, "PE": 0.0, "ACT": 0.0}
        self.no_pool = bool(os.environ.get("BD_NO_POOL"))
        self.no_pe = bool(os.environ.get("BD_NO_PE"))

    def charge(self, eng, ns):
        self.busy[eng] += ns

    def pick_op(self, elems, cands=("V", "P")):
        if self.no_pool:
            cands = ("V",)
        best, cost = None, None
        for e in cands:
            c = self.busy[e] + elems * self.RATE[e] + self.FIX[e]
            if cost is None or c < cost:
                best, cost = e, c
        self.busy[best] = cost
        return best

    def pick_pair(self, elems, allow_pe=True):
        if self.no_pe:
            allow_pe = False
        vc = self.busy["V"] + 2 * (elems * self.RATE["V"] + self.FIX["V"])
        pc = self.busy["P"] + 2 * (elems * self.RATE["P"] + self.FIX["P"])
        if self.no_pool:
            pc = vc + 1e9
        nchunk = (elems + 511) // 512
        pe_t = elems * self.PE_RATE + nchunk * self.PE_FIX
        act_t = elems * self.ACT_RATE + nchunk * self.ACT_FIX
        ec = max(self.busy["PE"] + pe_t, self.busy["ACT"] + act_t)
        if allow_pe and ec < vc and ec < pc:
            self.busy["PE"] += pe_t
            self.busy["ACT"] += act_t
            return "PE"
        if vc <= pc:
            self.busy["V"] = vc
            return "V"
        self.busy["P"] = pc
        return "P"


def _build():
    nc = bacc.Bacc(trn_type="TRN2")

    # Host-prepped compute-ready layouts (see _prep_core):
    #   hy_*[ci]  : [FS, NP*A*M] supertile image (planes 0..7 = H columns,
    #               plane 8 = y), free index = (j*A + i)*M + m, m = (u,b',s)
    #   out[ci,k] : [FS, 2*M] = (xr || xi) for matrix row k
    hyre = nc.dram_tensor("hy_re", [NCH, FS, NP * A * M], F32, kind="ExternalInput")
    hyim = nc.dram_tensor("hy_im", [NCH, FS, NP * A * M], F32, kind="ExternalInput")
    out = nc.dram_tensor("out", [NCH, A, FS, 2 * M], F32, kind="ExternalOutput")

    bal = _Balancer()

    with TileContext(nc) as tc:
        with (
            tc.tile_pool(name="consts", bufs=1) as consts,
            tc.tile_pool(name="supers", bufs=2) as supers,
            tc.tile_pool(name="work", bufs=1) as work,
            tc.tile_pool(name="fpool", bufs=2) as fpool,
            tc.tile_pool(name="prods", bufs=3) as prods,
            tc.tile_pool(name="xpool", bufs=2) as xpool,
            tc.tile_pool(name="pacc", bufs=6, space="PSUM") as pacc,
        ):
            ident = consts.tile([128, 128], F32)
            make_identity(nc, ident)
            negid = consts.tile([128, 128], F32)
            nc.vector.tensor_scalar_mul(negid, ident, -1.0)

            def emul(eng, o, a, b):
                (nc.vector if eng == "V" else nc.gpsimd).tensor_mul(o, a, b)

            def esub(eng, o, a, b):
                (nc.vector if eng == "V" else nc.gpsimd).tensor_sub(o, a, b)

            def pe_pair(dst, pa, pb, sa, sb, elems):
                """dst = dst (sa) pa (sb) pb via fp32 PSUM identity accumulate."""
                wa = negid if sa < 0 else ident
                wb = negid if sb < 0 else ident
                for off in range(0, elems, 512):
                    w = min(512, elems - off)
                    ps = pacc.tile([128, 512], F32, tag="pacc")
                    nc.tensor.matmul(
                        ps[:, :w], ident, dst[:, off : off + w],
                        start=True, stop=False,
                    )
                    nc.tensor.matmul(
                        ps[:, :w], wa, pa[:, off : off + w],
                        start=False, stop=False,
                    )
                    nc.tensor.matmul(
                        ps[:, :w], wb, pb[:, off : off + w],
                        start=False, stop=True,
                    )
                    nc.scalar.copy(dst[:, off : off + w], ps[:, :w])

            def combine(dst, pa, pb, sa, sb, elems, allow_pe=True):
                """dst = dst (sa) pa (sb) pb, signs in {+1,-1}."""
                eng = bal.pick_pair(elems, allow_pe=allow_pe)
                if eng == "PE":
                    pe_pair(dst, pa, pb, sa, sb, elems)
                    return
                ev = nc.vector if eng == "V" else nc.gpsimd
                (ev.tensor_add if sa > 0 else ev.tensor_sub)(dst, dst, pa)
                (ev.tensor_add if sb > 0 else ev.tensor_sub)(dst, dst, pb)

            for ci in range(NCH):
                HRe = supers.tile([128, NP * A * M], F32, tag="HRe")
                HIm = supers.tile([128, NP * A * M], F32, tag="HIm")

                def row(T, j, i):
                    return T[:, _off(j, i) : _off(j, i) + M]

                def rows2(T, j, i0, n):
                    base = _off(j, i0)
                    return T[:, base : base + n * M]

                def rows3(T, j, i0, n):
                    return rows2(T, j, i0, n).rearrange("p (r c) -> p r c", r=n)

                # ---------------- load (plain partition-major DMAs) --------
                for j0, j1 in ((0, 2), (2, 5), (5, 9)):
                    lo, hi = j0 * A * M, j1 * A * M
                    nc.sync.dma_start(HRe[:, lo:hi], hyre[ci, :, lo:hi])
                    nc.sync.dma_start(HIm[:, lo:hi], hyim[ci, :, lo:hi])

                # ---------------- solve ----------------
                INV = work.tile([128, 3 * A * M], F32, tag="INV")

                def inv_pair(k, n=None):
                    v = INV[:, k * M : k * M + 2 * A * M].rearrange(
                        "p (j c) -> p j c", j=2
                    )[:, :, :M]
                    if n is None:
                        return v
                    return v[:, :, None, :].broadcast_to([128, 2, n, M])

                fstate = {}

                def pivot_factors(k):
                    """Pivot reciprocal + elimination factors for step k
                    (critical path: pinned to DVE/ACT)."""
                    n = A - 1 - k
                    a = row(HRe, k, k)
                    b_ = row(HIm, k, k)
                    TD = fpool.tile([128, 3 * M], F32, tag="TD")
                    nc.scalar.square(TD[:, :M], a)
                    nc.scalar.square(TD[:, M : 2 * M], b_)
                    nc.vector.tensor_add(TD[:, :M], TD[:, :M], TD[:, M : 2 * M])
                    nc.vector.reciprocal(TD[:, 2 * M :], TD[:, :M])
                    irk = INV[:, k * M : (k + 1) * M]
                    iik = INV[:, (A + k) * M : (A + k + 1) * M]
                    nc.vector.tensor_mul(irk, a, TD[:, 2 * M :])
                    nc.vector.tensor_mul(iik, b_, TD[:, 2 * M :])
                    bal.charge("V", (4.2 * M) * bal.RATE["V"] + 4 * bal.FIX["V"])
                    bal.charge("ACT", 2 * (M * bal.ACT_RATE + bal.ACT_FIX))
                    if n == 0:
                        return
                    car = rows3(HRe, k, k + 1, n)
                    cai = rows3(HIm, k, k + 1, n)
                    car4 = car[:, None, :, :].broadcast_to([128, 2, n, M])
                    cai4 = cai[:, None, :, :].broadcast_to([128, 2, n, M])
                    PF1 = work.tile([128, 2 * (A - 1) * M], F32, tag="PF1")
                    PF2 = work.tile([128, 2 * (A - 1) * M], F32, tag="PF2")
                    F2 = fpool.tile([128, 2 * (A - 1) * M], F32, tag="F2")
                    p1v = PF1[:, : 2 * n * M].rearrange(
                        "p (j r c) -> p j r c", j=2, r=n
                    )
                    p2v = PF2[:, : 2 * n * M].rearrange(
                        "p (j r c) -> p j r c", j=2, r=n
                    )
                    emul("V", p1v, car4, inv_pair(k, n))
                    emul("V", p2v, cai4, inv_pair(k, n))
                    bal.charge("V", 4 * n * M * bal.RATE["V"] + 2 * bal.FIX["V"])
                    fre = F2[:, : n * M]
                    fim = F2[:, (A - 1) * M : (A - 1) * M + n * M]
                    # fre = -(a*ir + b*ii), fim = a*ii - b*ir
                    nc.vector.scalar_tensor_tensor(
                        fre, PF1[:, : n * M], -1.0,
                        PF2[:, n * M : 2 * n * M],
                        AL.mult, AL.subtract,
                    )
                    nc.vector.tensor_sub(
                        fim, PF1[:, n * M : 2 * n * M],
                        PF2[:, : n * M],
                    )
                    bal.charge("V", 2 * n * M * bal.RATE["V"] + 2 * bal.FIX["V"])
                    f3 = F2[:, : 2 * (A - 1) * M].rearrange(
                        "p (j c) -> p j c", j=2
                    )[:, :, : n * M]
                    fstate[k] = f3.rearrange("p j (r c) -> p j r c", r=n)

                def update_plane(k, j, critical=False):
                    """Eliminate column k from plane j (rows k+1..7)."""
                    n = A - 1 - k
                    f4 = fstate[k]
                    br = row(HRe, j, k)[:, None, None, :].broadcast_to(
                        [128, 2, n, M]
                    )
                    bi = row(HIm, j, k)[:, None, None, :].broadcast_to(
                        [128, 2, n, M]
                    )
                    PA = prods.tile([128, 2 * (A - 1) * M], F32, tag="PA")
                    PB = prods.tile([128, 2 * (A - 1) * M], F32, tag="PB")
                    pa4 = PA[:, : 2 * n * M].rearrange(
                        "p (j r c) -> p j r c", j=2, r=n
                    )
                    pb4 = PB[:, : 2 * n * M].rearrange(
                        "p (j r c) -> p j r c", j=2, r=n
                    )
                    if critical:
                        ea = eb = "V"
                        bal.charge(
                            "V", 8 * n * M * bal.RATE["V"] + 2 * bal.FIX["V"]
                        )
                    else:
                        ea = bal.pick_op(2 * n * M)
                        eb = bal.pick_op(2 * n * M)
                    emul(ea, pa4, f4, br)
                    emul(eb, pb4, f4, bi)
                    # hr_j += PA[0] - PB[1];  hi_j += PB[0] + PA[1]
                    hrj = rows2(HRe, j, k + 1, n)
                    hij = rows2(HIm, j, k + 1, n)
                    paR = PA[:, : n * M]
                    paI = PA[:, n * M : 2 * n * M]
                    pbR = PB[:, : n * M]
                    pbI = PB[:, n * M : 2 * n * M]
                    if critical:
                        nc.vector.tensor_add(hrj, hrj, paR)
                        nc.vector.tensor_sub(hrj, hrj, pbI)
                        nc.vector.tensor_add(hij, hij, pbR)
                        nc.vector.tensor_add(hij, hij, paI)
                    else:
                        combine(hrj, paR, pbI, +1, -1, n * M)
                        combine(hij, pbR, paI, +1, +1, n * M)

                # forward elimination, software-pipelined
                pivot_factors(0)
                for k in range(A - 1):
                    update_plane(k, k + 1, critical=True)
                    pivot_factors(k + 1)
                    for j in range(k + 2, NP):
                        update_plane(k, j)

                # back substitution (Jordan), pipelined: row k-1 cleared
                # first so x_{k-1} can start while rows 0..k-2 clear in bulk
                def solve_x(k):
                    yr = row(HRe, 8, k)
                    yi = row(HIm, 8, k)
                    X2 = xpool.tile([128, 2 * M], F32, tag="X2")
                    BP = xpool.tile([128, 4 * M], F32, tag="BP")
                    p1 = BP[:, : 2 * M].rearrange("p (j c) -> p j c", j=2)
                    p2 = BP[:, 2 * M :].rearrange("p (j c) -> p j c", j=2)
                    yr2 = yr[:, None, :].broadcast_to([128, 2, M])
                    yi2 = yi[:, None, :].broadcast_to([128, 2, M])
                    nc.vector.tensor_mul(p1, yr2, inv_pair(k))
                    nc.vector.tensor_mul(p2, yi2, inv_pair(k))
                    # xr = yr*ir + yi*ii, xi = yi*ir - yr*ii
                    nc.vector.tensor_add(X2[:, :M], BP[:, :M], BP[:, 3 * M :])
                    nc.vector.tensor_sub(
                        X2[:, M :], BP[:, 2 * M : 3 * M], BP[:, M : 2 * M]
                    )
                    bal.charge("V", 6 * M * bal.RATE["V"] + 4 * bal.FIX["V"])
                    return X2

                def clear_rows(k, X2, i0, nr, critical):
                    """y_i -= H[i,k] * x_k for i = i0..i0+nr-1."""
                    cr = rows3(HRe, k, i0, nr)[:, None, :, :].broadcast_to(
                        [128, 2, nr, M]
                    )
                    ci_ = rows3(HIm, k, i0, nr)[:, None, :, :].broadcast_to(
                        [128, 2, nr, M]
                    )
                    x4 = X2.rearrange("p (j c) -> p j c", j=2)[
                        :, :, None, :
                    ].broadcast_to([128, 2, nr, M])
                    QA = prods.tile([128, 2 * (A - 1) * M], F32, tag="PA")
                    QB = prods.tile([128, 2 * (A - 1) * M], F32, tag="PB")
                    qa4 = QA[:, : 2 * nr * M].rearrange(
                        "p (j r c) -> p j r c", j=2, r=nr
                    )
                    qb4 = QB[:, : 2 * nr * M].rearrange(
                        "p (j r c) -> p j r c", j=2, r=nr
                    )
                    # QA = (cr*xr, cr*xi), QB = (ci*xr, ci*xi)
                    if critical:
                        ea = eb = "V"
                        bal.charge(
                            "V", 8 * nr * M * bal.RATE["V"] + 6 * bal.FIX["V"]
                        )
                    else:
                        ea = bal.pick_op(2 * nr * M)
                        eb = bal.pick_op(2 * nr * M)
                    emul(ea, qa4, cr, x4)
                    emul(eb, qb4, ci_, x4)
                    ytr = rows2(HRe, 8, i0, nr)
                    yti = rows2(HIm, 8, i0, nr)
                    # ytr -= QA[0] - QB[1];  yti -= QA[1] + QB[0]
                    if critical:
                        nc.vector.tensor_sub(ytr, ytr, QA[:, : nr * M])
                        nc.vector.tensor_add(
                            ytr, ytr, QB[:, nr * M : 2 * nr * M]
                        )
                        nc.vector.tensor_sub(
                            yti, yti, QA[:, nr * M : 2 * nr * M]
                        )
                        nc.vector.tensor_sub(yti, yti, QB[:, : nr * M])
                    else:
                        combine(ytr, QA[:, : nr * M],
                                QB[:, nr * M : 2 * nr * M], -1, +1, nr * M)
                        combine(yti, QA[:, nr * M : 2 * nr * M],
                                QB[:, : nr * M], -1, -1, nr * M)

                for k in range(A - 1, -1, -1):
                    X2 = solve_x(k)
                    if k > 0:
                        clear_rows(k, X2, k - 1, 1, critical=True)
                    nc.sync.dma_start(out[ci, k], X2)
                    if k > 1:
                        clear_rows(k, X2, 0, k - 1, critical=False)

    nc.finalize()
    if os.environ.get("BD_DEBUG"):
        print("balancer busy (ns):", {k: round(v) for k, v in bal.busy.items()})
    return nc


_NC_CACHE = None


def _get_nc():
    global _NC_CACHE
    if _NC_CACHE is None:
        _NC_CACHE = _build()
    return _NC_CACHE


def _prep_core(y_re, y_im, h_re, h_im, c):
    """Host-side shard prep for core c: f-slice, block-diagonal extraction,
    and pre-transposition into the on-chip compute layout."""
    fsl = slice(c * FS, (c + 1) * FS)
    ue = np.arange(U)
    maps = {}
    for name, h, y in (("hy_re", h_re, y_re), ("hy_im", h_im, y_im)):
        h6 = h[:, 0, :, :, :, :, fsl].reshape(B, U, A, U, A, S, FS)
        hd = h6[:, ue, :, ue]                    # [u, b, i, j, s, f]
        hdt = hd.transpose(5, 3, 2, 0, 1, 4)     # [f, j, i, u, b, s]
        y5 = y[:, 0, :, :, fsl].reshape(B, U, A, S, FS)   # [b, u, i, s, f]
        yt = y5.transpose(4, 2, 1, 0, 3)         # [f, i, u, b, s]
        sup = np.empty((NCH, FS, NP, A, U, NB, S), np.float32)
        for ci in range(NCH):
            bsl = slice(ci * NB, (ci + 1) * NB)
            sup[ci, :, :A] = hdt[:, :, :, :, bsl]
            sup[ci, :, A] = yt[:, :, :, bsl]
        maps[name] = np.ascontiguousarray(sup.reshape(NCH, FS, NP * A * M))
    return maps


def kernel(y_re, y_im, h_re, h_im, **_ignored):
    global LAST_RESULTS
    y_re = np.asarray(y_re, dtype=np.float32)
    y_im = np.asarray(y_im, dtype=np.float32)
    h_re = np.asarray(h_re, dtype=np.float32)
    h_im = np.asarray(h_im, dtype=np.float32)

    nc = _get_nc()
    in_maps = [_prep_core(y_re, y_im, h_re, h_im, c) for c in range(NCORES)]
    trace = bool(int(os.environ.get("BD_TRACE", "0")))
    res = run_bass_kernel_spmd(
        nc, in_maps, core_ids=list(range(NCORES)), trace=trace
    )
    LAST_RESULTS = res
    outs = []
    for r in res.results:
        o = r["out"]                              # [ci, k, f, (c, u, b', s)]
        o = o.reshape(NCH, A, FS, 2, U, NB, S)
        o = o.transpose(0, 5, 4, 1, 6, 2, 3)      # [ci, b', u, k, s, f, c]
        o = o.reshape(B, U * A, S, FS, 2)         # [b, (u,i)=nr, s, f, c]
        outs.append(o)
    full = np.concatenate(outs, axis=3)           # [B, NR, S, F, 2]
    return np.ascontiguousarray(full[:, None])    # [B, 1, NR, S, F, 2]


# revision 18
# speedup vs baseline: 1.4687x; 1.0436x over previous
"""Block-diagonal ZF equalizer (nn_BDEqualizer) as a Trainium2 Bass kernel.

Math: for every resource element (b, s, f) and UE u, solve the 8x8 complex
system H_u x_u = y_u where H_u[i, j] = h[b, 0, 8u+i, u, j, s, f] and
y_u[i] = y[b, 0, 8u+i, s, f].  Output x as [B, 1, 32, S, F, 2] (re/im last).

Strategy (data-parallel over the fft axis, per the sharding hint):
  - 8 cores, each owns a contiguous 128-subcarrier slice of F=1024.
  - The host pre-extracts the block-diagonal channel blocks AND pre-
    transposes them into the exact on-chip compute layout (subcarriers on
    the 128 SBUF partitions, the (j-plane, i-row, u/b/s) RE axes along the
    free dim), so loads and stores are plain partition-major DMAs: no
    on-chip transposes, no staging, no PSUM drains on the load path.
  - Unpivoted complex Gaussian elimination on the 9-plane augmented
    supertile, fp32 throughout, software-pipelined at two levels: within
    a step (plane k+1 first on DVE - the critical path - then step k+1's
    pivot/factors while the bulk planes stream), and across the two
    b-chunks (chunk 0's serial back-substitution is emission-interleaved
    with chunk 1's forward elimination so the in-order engine queues
    never head-of-line block on the serial chain).
  - Three elementwise lanes, balanced at build time by a greedy cost
    tracker: DVE (1.04 ns/elem), Pool (1.98 ns/elem), and the Tensor
    engine as an adder lane - identity-weight fp32 matmuls accumulate
    (H + P1 +- P2) in PSUM (weight loads are free) with ScalarE draining
    the result back to SBUF.  Complex products are emitted as interleaved
    pairs ((fr||fi) * bcast(b)), one instruction covering two planes.
"""

import os

import numpy as np

import concourse.bacc as bacc
import concourse.mybir as mybir
from concourse.bass_utils import run_bass_kernel_spmd
from concourse.masks import make_identity
from concourse.tile import TileContext

B, NRX, NR, U, A, S, F = 4, 1, 32, 4, 8, 14, 1024
NCORES = 8
FS = F // NCORES        # 128 subcarriers per core
NB = 2                  # batch entries per chunk
NCH = B // NB           # chunks per core
M = U * NB * S          # 112 RE columns per chunk (u, b, s)
NP = 9                  # augmented planes: 8 matrix columns + rhs
F32 = mybir.dt.float32
AL = mybir.AluOpType

LAST_RESULTS = None     # BassKernelResults of the most recent run (for test.py)


def _off(j, i):
    """Free-dim offset of (plane j, row i) inside an H supertile."""
    return (j * A + i) * M


class _Balancer:
    """Greedy build-time engine load balancer."""

    RATE = {"V": 1.042, "P": 1.984}
    FIX = {"V": 62.0, "P": 8.0}
    PE_RATE = 3 * 1.8           # 3 fp32 matmul rows per pair elem
    PE_FIX = 30.0
    ACT_RATE = 0.833
    ACT_FIX = 150.0

    def __init__(self):
        self.busy = {"V": 0.0, "P": 0.0, "PE": 0.0, "ACT": 0.0}
        self.no_pool = bool(os.environ.get("BD_NO_POOL"))
        self.no_pe = bool(os.environ.get("BD_NO_PE"))

    def charge(self, eng, ns):
        self.busy[eng] += ns

    def pick_op(self, elems, cands=("V", "P")):
        if self.no_pool:
            cands = ("V",)
        best, cost = None, None
        for e in cands:
            c = self.busy[e] + elems * self.RATE[e] + self.FIX[e]
            if cost is None or c < cost:
                best, cost = e, c
        self.busy[best] = cost
        return best

    def pick_pair(self, elems, allow_pe=True):
        if self.no_pe:
            allow_pe = False
        vc = self.busy["V"] + 2 * (elems * self.RATE["V"] + self.FIX["V"])
        pc = self.busy["P"] + 2 * (elems * self.RATE["P"] + self.FIX["P"])
        if self.no_pool:
            pc = vc + 1e9
        nchunk = (elems + 511) // 512
        pe_t = elems * self.PE_RATE + nchunk * self.PE_FIX
        act_t = elems * self.ACT_RATE + nchunk * self.ACT_FIX
        ec = max(self.busy["PE"] + pe_t, self.busy["ACT"] + act_t)
        if allow_pe and ec < vc and ec < pc:
            self.busy["PE"] += pe_t
            self.busy["ACT"] += act_t
            return "PE"
        if vc <= pc:
            self.busy["V"] = vc
            return "V"
        self.busy["P"] = pc
        return "P"


def _drive(*gens):
    """Round-robin the emission generators until all are exhausted."""
    live = list(gens)
    while live:
        for g in list(live):
            try:
                next(g)
            except StopIteration:
                live.remove(g)


def _build():
    nc = bacc.Bacc(trn_type="TRN2")

    # Host-prepped compute-ready layouts (see _prep_core):
    #   hy_*[ci]  : [FS, NP*A*M] supertile image (planes 0..7 = H columns,
    #               plane 8 = y), free index = (j*A + i)*M + m, m = (u,b',s)
    #   out[ci,k] : [FS, 2*M] = (xr || xi) for matrix row k
    hyre = nc.dram_tensor("hy_re", [NCH, FS, NP * A * M], F32, kind="ExternalInput")
    hyim = nc.dram_tensor("hy_im", [NCH, FS, NP * A * M], F32, kind="ExternalInput")
    out = nc.dram_tensor("out", [NCH, A, FS, 2 * M], F32, kind="ExternalOutput")

    bal = _Balancer()

    with TileContext(nc) as tc:
        with (
            tc.tile_pool(name="consts", bufs=1) as consts,
            tc.tile_pool(name="supers", bufs=2) as supers,
            tc.tile_pool(name="work", bufs=1) as work,
            tc.tile_pool(name="invp", bufs=2) as invp,
            tc.tile_pool(name="fpool", bufs=2) as fpool,
            tc.tile_pool(name="prods", bufs=2) as prods,
            tc.tile_pool(name="xpool", bufs=2) as xpool,
            tc.tile_pool(name="pacc", bufs=6, space="PSUM") as pacc,
        ):
            ident = consts.tile([128, 128], F32)
            make_identity(nc, ident)
            negid = consts.tile([128, 128], F32)
            nc.vector.tensor_scalar_mul(negid, ident, -1.0)

            def emul(eng, o, a, b):
                (nc.vector if eng == "V" else nc.gpsimd).tensor_mul(o, a, b)

            def pe_pair(dst, pa, pb, sa, sb, elems):
                """dst = dst (sa) pa (sb) pb via fp32 PSUM identity accumulate."""
                wa = negid if sa < 0 else ident
                wb = negid if sb < 0 else ident
                for off in range(0, elems, 512):
                    w = min(512, elems - off)
                    ps = pacc.tile([128, 512], F32, tag="pacc")
                    nc.tensor.matmul(
                        ps[:, :w], ident, dst[:, off : off + w],
                        start=True, stop=False,
                    )
                    nc.tensor.matmul(
                        ps[:, :w], wa, pa[:, off : off + w],
                        start=False, stop=False,
                    )
                    nc.tensor.matmul(
                        ps[:, :w], wb, pb[:, off : off + w],
                        start=False, stop=True,
                    )
                    nc.scalar.copy(dst[:, off : off + w], ps[:, :w])

            def combine(dst, pa, pb, sa, sb, elems, allow_pe=True):
                """dst = dst (sa) pa (sb) pb, signs in {+1,-1}."""
                eng = bal.pick_pair(elems, allow_pe=allow_pe)
                if eng == "PE":
                    pe_pair(dst, pa, pb, sa, sb, elems)
                    return
                ev = nc.vector if eng == "V" else nc.gpsimd
                (ev.tensor_add if sa > 0 else ev.tensor_sub)(dst, dst, pa)
                (ev.tensor_add if sb > 0 else ev.tensor_sub)(dst, dst, pb)

            # ---------------- per-chunk state + loads ----------------
            states = []
            for ci in range(NCH):
                st = {
                    "ci": ci,
                    "HRe": supers.tile(
                        [128, NP * A * M], F32, tag="HRe", name=f"HRe{ci}"
                    ),
                    "HIm": supers.tile(
                        [128, NP * A * M], F32, tag="HIm", name=f"HIm{ci}"
                    ),
                    "INV": invp.tile(
                        [128, 3 * A * M], F32, tag="INV", name=f"INV{ci}"
                    ),
                    "fstate": {},
                }
                states.append(st)
            for st in states:
                ci = st["ci"]
                for j0, j1 in ((0, 1), (1, 2), (2, 5), (5, 9)):
                    lo, hi = j0 * A * M, j1 * A * M
                    nc.sync.dma_start(st["HRe"][:, lo:hi], hyre[ci, :, lo:hi])
                    nc.sync.dma_start(st["HIm"][:, lo:hi], hyim[ci, :, lo:hi])

            def row(T, j, i):
                return T[:, _off(j, i) : _off(j, i) + M]

            def rows2(T, j, i0, n):
                base = _off(j, i0)
                return T[:, base : base + n * M]

            def rows3(T, j, i0, n):
                return rows2(T, j, i0, n).rearrange("p (r c) -> p r c", r=n)

            def inv_pair(st, k, n=None):
                v = st["INV"][:, k * M : k * M + 2 * A * M].rearrange(
                    "p (j c) -> p j c", j=2
                )[:, :, :M]
                if n is None:
                    return v
                return v[:, :, None, :].broadcast_to([128, 2, n, M])

            def pivot_factors(st, k):
                """Pivot reciprocal + elimination factors for step k
                (critical path: pinned to DVE/ACT)."""
                HRe, HIm, INV = st["HRe"], st["HIm"], st["INV"]
                n = A - 1 - k
                a = row(HRe, k, k)
                b_ = row(HIm, k, k)
                TD = fpool.tile([128, 3 * M], F32, tag="TD")
                nc.scalar.square(TD[:, :M], a)
                nc.scalar.square(TD[:, M : 2 * M], b_)
                nc.vector.tensor_add(TD[:, :M], TD[:, :M], TD[:, M : 2 * M])
                nc.vector.reciprocal(TD[:, 2 * M :], TD[:, :M])
                irk = INV[:, k * M : (k + 1) * M]
                iik = INV[:, (A + k) * M : (A + k + 1) * M]
                nc.vector.tensor_mul(irk, a, TD[:, 2 * M :])
                nc.vector.tensor_mul(iik, b_, TD[:, 2 * M :])
                bal.charge("V", (4.2 * M) * bal.RATE["V"] + 4 * bal.FIX["V"])
                bal.charge("ACT", 2 * (M * bal.ACT_RATE + bal.ACT_FIX))
                if n == 0:
                    return
                car = rows3(HRe, k, k + 1, n)
                cai = rows3(HIm, k, k + 1, n)
                car4 = car[:, None, :, :].broadcast_to([128, 2, n, M])
                cai4 = cai[:, None, :, :].broadcast_to([128, 2, n, M])
                PF1 = work.tile([128, 2 * (A - 1) * M], F32, tag="PF1")
                PF2 = work.tile([128, 2 * (A - 1) * M], F32, tag="PF2")
                F2 = fpool.tile([128, 2 * (A - 1) * M], F32, tag="F2")
                p1v = PF1[:, : 2 * n * M].rearrange(
                    "p (j r c) -> p j r c", j=2, r=n
                )
                p2v = PF2[:, : 2 * n * M].rearrange(
                    "p (j r c) -> p j r c", j=2, r=n
                )
                emul("V", p1v, car4, inv_pair(st, k, n))
                emul("V", p2v, cai4, inv_pair(st, k, n))
                bal.charge("V", 4 * n * M * bal.RATE["V"] + 2 * bal.FIX["V"])
                fre = F2[:, : n * M]
                fim = F2[:, (A - 1) * M : (A - 1) * M + n * M]
                # fre = -(a*ir + b*ii), fim = a*ii - b*ir
                nc.vector.scalar_tensor_tensor(
                    fre, PF1[:, : n * M], -1.0,
                    PF2[:, n * M : 2 * n * M],
                    AL.mult, AL.subtract,
                )
                nc.vector.tensor_sub(
                    fim, PF1[:, n * M : 2 * n * M], PF2[:, : n * M]
                )
                bal.charge("V", 2 * n * M * bal.RATE["V"] + 2 * bal.FIX["V"])
                f3 = F2[:, : 2 * (A - 1) * M].rearrange(
                    "p (j c) -> p j c", j=2
                )[:, :, : n * M]
                st["fstate"][k] = f3.rearrange("p j (r c) -> p j r c", r=n)

            def update_plane(st, k, j, critical=False):
                """Eliminate column k from plane j (rows k+1..7)."""
                HRe, HIm = st["HRe"], st["HIm"]
                n = A - 1 - k
                f4 = st["fstate"][k]
                br = row(HRe, j, k)[:, None, None, :].broadcast_to(
                    [128, 2, n, M]
                )
                bi = row(HIm, j, k)[:, None, None, :].broadcast_to(
                    [128, 2, n, M]
                )
                PA = prods.tile([128, 2 * (A - 1) * M], F32, tag="PA")
                PB = prods.tile([128, 2 * (A - 1) * M], F32, tag="PB")
                pa4 = PA[:, : 2 * n * M].rearrange(
                    "p (j r c) -> p j r c", j=2, r=n
                )
                pb4 = PB[:, : 2 * n * M].rearrange(
                    "p (j r c) -> p j r c", j=2, r=n
                )
                if critical:
                    ea = eb = "V"
                    bal.charge("V", 8 * n * M * bal.RATE["V"] + 2 * bal.FIX["V"])
                else:
                    ea = bal.pick_op(2 * n * M)
                    eb = bal.pick_op(2 * n * M)
                emul(ea, pa4, f4, br)
                emul(eb, pb4, f4, bi)
                # hr_j += PA[0] - PB[1];  hi_j += PB[0] + PA[1]
                hrj = rows2(HRe, j, k + 1, n)
                hij = rows2(HIm, j, k + 1, n)
                paR = PA[:, : n * M]
                paI = PA[:, n * M : 2 * n * M]
                pbR = PB[:, : n * M]
                pbI = PB[:, n * M : 2 * n * M]
                if critical:
                    nc.vector.tensor_add(hrj, hrj, paR)
                    nc.vector.tensor_sub(hrj, hrj, pbI)
                    nc.vector.tensor_add(hij, hij, pbR)
                    nc.vector.tensor_add(hij, hij, paI)
                else:
                    combine(hrj, paR, pbI, +1, -1, n * M)
                    combine(hij, pbR, paI, +1, +1, n * M)

            def solve_x(st, k):
                HRe, HIm = st["HRe"], st["HIm"]
                yr = row(HRe, 8, k)
                yi = row(HIm, 8, k)
                X2 = xpool.tile([128, 2 * M], F32, tag="X2")
                BP = xpool.tile([128, 4 * M], F32, tag="BP")
                p1 = BP[:, : 2 * M].rearrange("p (j c) -> p j c", j=2)
                p2 = BP[:, 2 * M :].rearrange("p (j c) -> p j c", j=2)
                yr2 = yr[:, None, :].broadcast_to([128, 2, M])
                yi2 = yi[:, None, :].broadcast_to([128, 2, M])
                nc.vector.tensor_mul(p1, yr2, inv_pair(st, k))
                nc.vector.tensor_mul(p2, yi2, inv_pair(st, k))
                # xr = yr*ir + yi*ii, xi = yi*ir - yr*ii
                nc.vector.tensor_add(X2[:, :M], BP[:, :M], BP[:, 3 * M :])
                nc.vector.tensor_sub(
                    X2[:, M :], BP[:, 2 * M : 3 * M], BP[:, M : 2 * M]
                )
                bal.charge("V", 6 * M * bal.RATE["V"] + 4 * bal.FIX["V"])
                return X2

            def clear_rows(st, k, X2, i0, nr, critical):
                """y_i -= H[i,k] * x_k for i = i0..i0+nr-1."""
                HRe, HIm = st["HRe"], st["HIm"]
                cr = rows3(HRe, k, i0, nr)[:, None, :, :].broadcast_to(
                    [128, 2, nr, M]
                )
                ci_ = rows3(HIm, k, i0, nr)[:, None, :, :].broadcast_to(
                    [128, 2, nr, M]
                )
                x4 = X2.rearrange("p (j c) -> p j c", j=2)[
                    :, :, None, :
                ].broadcast_to([128, 2, nr, M])
                QA = prods.tile([128, 2 * (A - 1) * M], F32, tag="PA")
                QB = prods.tile([128, 2 * (A - 1) * M], F32, tag="PB")
                qa4 = QA[:, : 2 * nr * M].rearrange(
                    "p (j r c) -> p j r c", j=2, r=nr
                )
                qb4 = QB[:, : 2 * nr * M].rearrange(
                    "p (j r c) -> p j r c", j=2, r=nr
                )
                # QA = (cr*xr, cr*xi), QB = (ci*xr, ci*xi)
                if critical:
                    ea = eb = "V"
                    bal.charge("V", 8 * nr * M * bal.RATE["V"] + 6 * bal.FIX["V"])
                else:
                    ea = bal.pick_op(2 * nr * M)
                    eb = bal.pick_op(2 * nr * M)
                emul(ea, qa4, cr, x4)
                emul(eb, qb4, ci_, x4)
                ytr = rows2(HRe, 8, i0, nr)
                yti = rows2(HIm, 8, i0, nr)
                # ytr -= QA[0] - QB[1];  yti -= QA[1] + QB[0]
                if critical:
                    nc.vector.tensor_sub(ytr, ytr, QA[:, : nr * M])
                    nc.vector.tensor_add(ytr, ytr, QB[:, nr * M : 2 * nr * M])
                    nc.vector.tensor_sub(yti, yti, QA[:, nr * M : 2 * nr * M])
                    nc.vector.tensor_sub(yti, yti, QB[:, : nr * M])
                else:
                    combine(ytr, QA[:, : nr * M],
                            QB[:, nr * M : 2 * nr * M], -1, +1, nr * M)
                    combine(yti, QA[:, nr * M : 2 * nr * M],
                            QB[:, : nr * M], -1, -1, nr * M)

            def fwd_gen(st):
                pivot_factors(st, 0)
                yield
                for k in range(A - 1):
                    update_plane(st, k, k + 1, critical=True)
                    yield
                    pivot_factors(st, k + 1)
                    yield
                    for j in range(k + 2, NP):
                        update_plane(st, k, j)
                        yield

            def back_gen(st):
                ci = st["ci"]
                for k in range(A - 1, -1, -1):
                    X2 = solve_x(st, k)
                    yield
                    if k > 0:
                        clear_rows(st, k, X2, k - 1, 1, critical=True)
                        yield
                    nc.sync.dma_start(out[ci, k], X2)
                    yield
                    if k > 1:
                        clear_rows(st, k, X2, 0, k - 1, critical=False)
                        yield

            _drive(fwd_gen(states[0]))
            _drive(back_gen(states[0]), fwd_gen(states[1]))
            _drive(back_gen(states[1]))

    nc.finalize()
    if os.environ.get("BD_DEBUG"):
        print("balancer busy (ns):", {k: round(v) for k, v in bal.busy.items()})
    return nc


_NC_CACHE = None


def _get_nc():
    global _NC_CACHE
    if _NC_CACHE is None:
        _NC_CACHE = _build()
    return _NC_CACHE


def _prep_core(y_re, y_im, h_re, h_im, c):
    """Host-side shard prep for core c: f-slice, block-diagonal extraction,
    and pre-transposition into the on-chip compute layout."""
    fsl = slice(c * FS, (c + 1) * FS)
    ue = np.arange(U)
    maps = {}
    for name, h, y in (("hy_re", h_re, y_re), ("hy_im", h_im, y_im)):
        h6 = h[:, 0, :, :, :, :, fsl].reshape(B, U, A, U, A, S, FS)
        hd = h6[:, ue, :, ue]                    # [u, b, i, j, s, f]
        hdt = hd.transpose(5, 3, 2, 0, 1, 4)     # [f, j, i, u, b, s]
        y5 = y[:, 0, :, :, fsl].reshape(B, U, A, S, FS)   # [b, u, i, s, f]
        yt = y5.transpose(4, 2, 1, 0, 3)         # [f, i, u, b, s]
        sup = np.empty((NCH, FS, NP, A, U, NB, S), np.float32)
        for ci in range(NCH):
            bsl = slice(ci * NB, (ci + 1) * NB)
            sup[ci, :, :A] = hdt[:, :, :, :, bsl]
            sup[ci, :, A] = yt[:, :, :, bsl]
        maps[name] = np.ascontiguousarray(sup.reshape(NCH, FS, NP * A * M))
    return maps


def kernel(y_re, y_im, h_re, h_im, **_ignored):
    global LAST_RESULTS
    y_re = np.asarray(y_re, dtype=np.float32)
    y_im = np.asarray(y_im, dtype=np.float32)
    h_re = np.asarray(h_re, dtype=np.float32)
    h_im = np.asarray(h_im, dtype=np.float32)

    nc = _get_nc()
    in_maps = [_prep_core(y_re, y_im, h_re, h_im, c) for c in range(NCORES)]
    trace = bool(int(os.environ.get("BD_TRACE", "0")))
    res = run_bass_kernel_spmd(
        nc, in_maps, core_ids=list(range(NCORES)), trace=trace
    )
    LAST_RESULTS = res
    outs = []
    for r in res.results:
        o = r["out"]                              # [ci, k, f, (c, u, b', s)]
        o = o.reshape(NCH, A, FS, 2, U, NB, S)
        o = o.transpose(0, 5, 4, 1, 6, 2, 3)      # [ci, b', u, k, s, f, c]
        o = o.reshape(B, U * A, S, FS, 2)         # [b, (u,i)=nr, s, f, c]
        outs.append(o)
    full = np.concatenate(outs, axis=3)           # [B, NR, S, F, 2]
    return np.ascontiguousarray(full[:, None])    # [B, 1, NR, S, F, 2]


# revision 27
# speedup vs baseline: 1.4937x; 1.0170x over previous
"""Block-diagonal ZF equalizer (nn_BDEqualizer) as a Trainium2 Bass kernel.

Math: for every resource element (b, s, f) and UE u, solve the 8x8 complex
system H_u x_u = y_u where H_u[i, j] = h[b, 0, 8u+i, u, j, s, f] and
y_u[i] = y[b, 0, 8u+i, s, f].  Output x as [B, 1, 32, S, F, 2] (re/im last).

Strategy (data-parallel over the fft axis, per the sharding hint):
  - 8 cores, each owns a contiguous 128-subcarrier slice of F=1024.
  - The host pre-extracts the block-diagonal channel blocks AND pre-
    transposes them into the exact on-chip compute layout (subcarriers on
    the 128 SBUF partitions, the (j-plane, i-row, u/b/s) RE axes along the
    free dim), so loads and stores are plain partition-major DMAs: no
    on-chip transposes, no staging, no PSUM drains on the load path.
  - Unpivoted complex Gaussian elimination on the 9-plane augmented
    supertile, fp32 throughout, software-pipelined at two levels: within
    a step (plane k+1 first on DVE - the critical path - then step k+1's
    pivot/factors while the bulk planes stream), and across the two
    b-chunks (chunk 0's serial back-substitution is emission-interleaved
    with chunk 1's forward elimination so the in-order engine queues
    never head-of-line block on the serial chain).
  - Three elementwise lanes, balanced at build time by a greedy cost
    tracker: DVE (1.04 ns/elem), Pool (1.98 ns/elem), and the Tensor
    engine as an adder lane - identity-weight fp32 matmuls accumulate
    (H + P1 +- P2) in PSUM (weight loads are free) with ScalarE draining
    the result back to SBUF.  Complex products are emitted as interleaved
    pairs ((fr||fi) * bcast(b)), one instruction covering two planes.
"""

import os

import numpy as np

import concourse.bacc as bacc
import concourse.mybir as mybir
from concourse.bass_utils import run_bass_kernel_spmd
from concourse.masks import make_identity
from concourse.tile import TileContext

B, NRX, NR, U, A, S, F = 4, 1, 32, 4, 8, 14, 1024
NCORES = 8
FS = F // NCORES        # 128 subcarriers per core
NB = 2                  # batch entries per chunk
NCH = B // NB           # chunks per core
M = U * NB * S          # 112 RE columns per chunk (u, b, s)
NP = 9                  # augmented planes: 8 matrix columns + rhs
F32 = mybir.dt.float32
AL = mybir.AluOpType

LAST_RESULTS = None     # BassKernelResults of the most recent run (for test.py)


def _off(j, i):
    """Free-dim offset of (plane j, row i) inside an H supertile."""
    return (j * A + i) * M


class _Balancer:
    """Greedy build-time engine load balancer."""

    RATE = {"V": 1.042, "P": 1.984}
    FIX = {"V": 62.0, "P": 8.0}
    PE_RATE = 3 * 1.8           # 3 fp32 matmul rows per pair elem
    PE_FIX = 30.0
    ACT_RATE = 0.833            # drain per pair elem
    ACT_FIX = 150.0

    def __init__(self):
        self.busy = {"V": 0.0, "P": 0.0, "PE": 0.0, "ACT": 0.0}
        self.no_pool = bool(os.environ.get("BD_NO_POOL"))
        self.no_pe = bool(os.environ.get("BD_NO_PE"))

    def charge(self, eng, ns):
        self.busy[eng] += ns

    def pick_op(self, elems, cands=("V", "P")):
        if self.no_pool:
            cands = ("V",)
        best, cost = None, None
        for e in cands:
            c = self.busy[e] + elems * self.RATE[e] + self.FIX[e]
            if cost is None or c < cost:
                best, cost = e, c
        self.busy[best] = cost
        return best

    def pick_pair(self, elems, allow_pe=True):
        if self.no_pe:
            allow_pe = False
        vc = self.busy["V"] + 2 * (elems * self.RATE["V"] + self.FIX["V"])
        pc = self.busy["P"] + 2 * (elems * self.RATE["P"] + self.FIX["P"])
        if self.no_pool:
            pc = vc + 1e9
        nchunk = (elems + 511) // 512
        pe_t = elems * self.PE_RATE + nchunk * self.PE_FIX
        act_t = elems * self.ACT_RATE + nchunk * self.ACT_FIX
        ec = max(self.busy["PE"] + pe_t, self.busy["ACT"] + act_t)
        if allow_pe and ec < vc and ec < pc:
            self.busy["PE"] += pe_t
            self.busy["ACT"] += act_t
            return "PE"
        if vc <= pc:
            self.busy["V"] = vc
            return "V"
        self.busy["P"] = pc
        return "P"


def _drive(*gens):
    """Round-robin the emission generators until all are exhausted."""
    live = list(gens)
    while live:
        for g in list(live):
            try:
                next(g)
            except StopIteration:
                live.remove(g)


def _build():
    nc = bacc.Bacc(trn_type="TRN2")

    # Host-prepped compute-ready layouts (see _prep_core):
    #   hy_*[ci]  : [FS, NP*A*M] supertile image (planes 0..7 = H columns,
    #               plane 8 = y), free index = (j*A + i)*M + m, m = (u,b',s)
    #   out[ci,k] : [FS, 2*M] = (xr || xi) for matrix row k
    hyre = nc.dram_tensor("hy_re", [NCH, FS, NP * A * M], F32, kind="ExternalInput")
    hyim = nc.dram_tensor("hy_im", [NCH, FS, NP * A * M], F32, kind="ExternalInput")
    out = nc.dram_tensor("out", [NCH, A, FS, 2 * M], F32, kind="ExternalOutput")

    bal = _Balancer()

    with TileContext(nc) as tc:
        with (
            tc.tile_pool(name="consts", bufs=1) as consts,
            tc.tile_pool(name="supers", bufs=2) as supers,
            tc.tile_pool(name="work", bufs=1) as work,
            tc.tile_pool(name="invp", bufs=2) as invp,
            tc.tile_pool(name="fpool", bufs=2) as fpool,
            tc.tile_pool(name="prods", bufs=3) as prods,
            tc.tile_pool(name="xpool", bufs=2) as xpool,
            tc.tile_pool(name="pacc", bufs=6, space="PSUM") as pacc,
        ):
            ident = consts.tile([128, 128], F32)
            make_identity(nc, ident)
            negid = consts.tile([128, 128], F32)
            nc.vector.tensor_scalar_mul(negid, ident, -1.0)

            def emul(eng, o, a, b):
                (nc.vector if eng == "V" else nc.gpsimd).tensor_mul(o, a, b)

            def pe_pair(dst, pa, pb, sa, sb, elems):
                """dst = dst (sa) pa (sb) pb via fp32 PSUM identity accumulate."""
                wa = negid if sa < 0 else ident
                wb = negid if sb < 0 else ident
                for off in range(0, elems, 512):
                    w = min(512, elems - off)
                    ps = pacc.tile([128, 512], F32, tag="pacc")
                    nc.tensor.matmul(
                        ps[:, :w], ident, dst[:, off : off + w],
                        start=True, stop=False,
                    )
                    nc.tensor.matmul(
                        ps[:, :w], wa, pa[:, off : off + w],
                        start=False, stop=False,
                    )
                    nc.tensor.matmul(
                        ps[:, :w], wb, pb[:, off : off + w],
                        start=False, stop=True,
                    )
                    nc.scalar.copy(dst[:, off : off + w], ps[:, :w])

            def combine(dst, pa, pb, sa, sb, elems, allow_pe=True):
                """dst = dst (sa) pa (sb) pb, signs in {+1,-1}."""
                eng = bal.pick_pair(elems, allow_pe=allow_pe)
                if eng == "PE":
                    pe_pair(dst, pa, pb, sa, sb, elems)
                    return
                ev = nc.vector if eng == "V" else nc.gpsimd
                (ev.tensor_add if sa > 0 else ev.tensor_sub)(dst, dst, pa)
                (ev.tensor_add if sb > 0 else ev.tensor_sub)(dst, dst, pb)

            # ---------------- per-chunk state + loads ----------------
            states = []
            for ci in range(NCH):
                st = {
                    "ci": ci,
                    "HRe": supers.tile(
                        [128, NP * A * M], F32, tag="HRe", name=f"HRe{ci}"
                    ),
                    "HIm": supers.tile(
                        [128, NP * A * M], F32, tag="HIm", name=f"HIm{ci}"
                    ),
                    "INV": invp.tile(
                        [128, 2 * A * M], F32, tag="INV", name=f"INV{ci}"
                    ),
                    "fstate": {},
                }
                states.append(st)
            for st in states:
                ci = st["ci"]
                for j0, j1 in ((0, 1), (1, 2), (2, 5), (5, 9)):
                    lo, hi = j0 * A * M, j1 * A * M
                    nc.sync.dma_start(st["HRe"][:, lo:hi], hyre[ci, :, lo:hi])
                    nc.sync.dma_start(st["HIm"][:, lo:hi], hyim[ci, :, lo:hi])

            def row(T, j, i):
                return T[:, _off(j, i) : _off(j, i) + M]

            def rows2(T, j, i0, n):
                base = _off(j, i0)
                return T[:, base : base + n * M]

            def rows3(T, j, i0, n):
                return rows2(T, j, i0, n).rearrange("p (r c) -> p r c", r=n)

            def inv_pair(st, k, n=None):
                v = st["INV"].rearrange("p (j c) -> p j c", j=2)[
                    :, :, k * M : (k + 1) * M
                ]
                if n is None:
                    return v
                return v[:, :, None, :].broadcast_to([128, 2, n, M])

            def pivot_factors(st, k):
                """Pivot reciprocal + elimination factors for step k
                (critical path: pinned to DVE/ACT)."""
                HRe, HIm, INV = st["HRe"], st["HIm"], st["INV"]
                n = A - 1 - k
                a = row(HRe, k, k)
                b_ = row(HIm, k, k)
                TD = work.tile([128, 3 * M], F32, tag="TD")
                nc.scalar.square(TD[:, :M], a)
                nc.scalar.square(TD[:, M : 2 * M], b_)
                nc.vector.tensor_add(TD[:, :M], TD[:, :M], TD[:, M : 2 * M])
                nc.vector.reciprocal(TD[:, 2 * M :], TD[:, :M])
                irk = INV[:, k * M : (k + 1) * M]
                iik = INV[:, (A + k) * M : (A + k + 1) * M]
                nc.vector.tensor_mul(irk, a, TD[:, 2 * M :])
                nc.vector.tensor_mul(iik, b_, TD[:, 2 * M :])
                bal.charge("V", (4.2 * M) * bal.RATE["V"] + 4 * bal.FIX["V"])
                bal.charge("ACT", 2 * (M * bal.ACT_RATE + bal.ACT_FIX))
                if n == 0:
                    return
                car = rows3(HRe, k, k + 1, n)
                cai = rows3(HIm, k, k + 1, n)
                car4 = car[:, None, :, :].broadcast_to([128, 2, n, M])
                cai4 = cai[:, None, :, :].broadcast_to([128, 2, n, M])
                PF1 = work.tile([128, 2 * (A - 1) * M], F32, tag="PF1")
                PF2 = work.tile([128, 2 * (A - 1) * M], F32, tag="PF2")
                F2 = fpool.tile([128, 2 * (A - 1) * M], F32, tag="F2")
                p1v = PF1[:, : 2 * n * M].rearrange(
                    "p (j r c) -> p j r c", j=2, r=n
                )
                p2v = PF2[:, : 2 * n * M].rearrange(
                    "p (j r c) -> p j r c", j=2, r=n
                )
                emul("V", p1v, car4, inv_pair(st, k, n))
                emul("V", p2v, cai4, inv_pair(st, k, n))
                bal.charge("V", 4 * n * M * bal.RATE["V"] + 2 * bal.FIX["V"])
                fre = F2[:, : n * M]
                fim = F2[:, (A - 1) * M : (A - 1) * M + n * M]
                # fre = -(a*ir + b*ii), fim = a*ii - b*ir
                nc.vector.scalar_tensor_tensor(
                    fre, PF1[:, : n * M], -1.0,
                    PF2[:, n * M : 2 * n * M],
                    AL.mult, AL.subtract,
                )
                nc.vector.tensor_sub(
                    fim, PF1[:, n * M : 2 * n * M], PF2[:, : n * M]
                )
                bal.charge("V", 2 * n * M * bal.RATE["V"] + 2 * bal.FIX["V"])
                f3 = F2[:, : 2 * (A - 1) * M].rearrange(
                    "p (j c) -> p j c", j=2
                )[:, :, : n * M]
                st["fstate"][k] = f3.rearrange("p j (r c) -> p j r c", r=n)

            def update_plane(st, k, j, critical=False):
                """Eliminate column k from plane j (rows k+1..7)."""
                HRe, HIm = st["HRe"], st["HIm"]
                n = A - 1 - k
                f4 = st["fstate"][k]
                br = row(HRe, j, k)[:, None, None, :].broadcast_to(
                    [128, 2, n, M]
                )
                bi = row(HIm, j, k)[:, None, None, :].broadcast_to(
                    [128, 2, n, M]
                )
                PA = prods.tile([128, 2 * (A - 1) * M], F32, tag="PA")
                PB = prods.tile([128, 2 * (A - 1) * M], F32, tag="PB")
                pa4 = PA[:, : 2 * n * M].rearrange(
                    "p (j r c) -> p j r c", j=2, r=n
                )
                pb4 = PB[:, : 2 * n * M].rearrange(
                    "p (j r c) -> p j r c", j=2, r=n
                )
                if critical:
                    ea = eb = "V"
                    bal.charge("V", 8 * n * M * bal.RATE["V"] + 2 * bal.FIX["V"])
                else:
                    ea = bal.pick_op(2 * n * M)
                    eb = bal.pick_op(2 * n * M)
                emul(ea, pa4, f4, br)
                emul(eb, pb4, f4, bi)
                # hr_j += PA[0] - PB[1];  hi_j += PB[0] + PA[1]
                hrj = rows2(HRe, j, k + 1, n)
                hij = rows2(HIm, j, k + 1, n)
                paR = PA[:, : n * M]
                paI = PA[:, n * M : 2 * n * M]
                pbR = PB[:, : n * M]
                pbI = PB[:, n * M : 2 * n * M]
                if critical:
                    nc.vector.tensor_add(hrj, hrj, paR)
                    nc.vector.tensor_sub(hrj, hrj, pbI)
                    nc.vector.tensor_add(hij, hij, pbR)
                    nc.vector.tensor_add(hij, hij, paI)
                else:
                    combine(hrj, paR, pbI, +1, -1, n * M)
                    combine(hij, pbR, paI, +1, +1, n * M)

            def solve_x(st, k):
                HRe, HIm = st["HRe"], st["HIm"]
                yr = row(HRe, 8, k)
                yi = row(HIm, 8, k)
                X2 = xpool.tile([128, 2 * M], F32, tag="X2")
                BP = work.tile([128, 4 * M], F32, tag="BP")
                p1 = BP[:, : 2 * M].rearrange("p (j c) -> p j c", j=2)
                p2 = BP[:, 2 * M :].rearrange("p (j c) -> p j c", j=2)
                yr2 = yr[:, None, :].broadcast_to([128, 2, M])
                yi2 = yi[:, None, :].broadcast_to([128, 2, M])
                nc.vector.tensor_mul(p1, yr2, inv_pair(st, k))
                nc.vector.tensor_mul(p2, yi2, inv_pair(st, k))
                # xr = yr*ir + yi*ii, xi = yi*ir - yr*ii
                nc.vector.tensor_add(X2[:, :M], BP[:, :M], BP[:, 3 * M :])
                nc.vector.tensor_sub(
                    X2[:, M :], BP[:, 2 * M : 3 * M], BP[:, M : 2 * M]
                )
                bal.charge("V", 6 * M * bal.RATE["V"] + 4 * bal.FIX["V"])
                return X2

            def clear_rows(st, k, X2, i0, nr, critical):
                """y_i -= H[i,k] * x_k for i = i0..i0+nr-1."""
                HRe, HIm = st["HRe"], st["HIm"]
                cr = rows3(HRe, k, i0, nr)[:, None, :, :].broadcast_to(
                    [128, 2, nr, M]
                )
                ci_ = rows3(HIm, k, i0, nr)[:, None, :, :].broadcast_to(
                    [128, 2, nr, M]
                )
                x4 = X2.rearrange("p (j c) -> p j c", j=2)[
                    :, :, None, :
                ].broadcast_to([128, 2, nr, M])
                QA = prods.tile([128, 2 * (A - 1) * M], F32, tag="PA")
                QB = prods.tile([128, 2 * (A - 1) * M], F32, tag="PB")
                qa4 = QA[:, : 2 * nr * M].rearrange(
                    "p (j r c) -> p j r c", j=2, r=nr
                )
                qb4 = QB[:, : 2 * nr * M].rearrange(
                    "p (j r c) -> p j r c", j=2, r=nr
                )
                # QA = (cr*xr, cr*xi), QB = (ci*xr, ci*xi)
                if critical:
                    ea = eb = "V"
                    bal.charge("V", 8 * nr * M * bal.RATE["V"] + 6 * bal.FIX["V"])
                else:
                    ea = bal.pick_op(2 * nr * M)
                    eb = bal.pick_op(2 * nr * M)
                emul(ea, qa4, cr, x4)
                emul(eb, qb4, ci_, x4)
                ytr = rows2(HRe, 8, i0, nr)
                yti = rows2(HIm, 8, i0, nr)
                # ytr -= QA[0] - QB[1];  yti -= QA[1] + QB[0]
                if critical:
                    nc.vector.tensor_sub(ytr, ytr, QA[:, : nr * M])
                    nc.vector.tensor_add(ytr, ytr, QB[:, nr * M : 2 * nr * M])
                    nc.vector.tensor_sub(yti, yti, QA[:, nr * M : 2 * nr * M])
                    nc.vector.tensor_sub(yti, yti, QB[:, : nr * M])
                else:
                    combine(ytr, QA[:, : nr * M],
                            QB[:, nr * M : 2 * nr * M], -1, +1, nr * M)
                    combine(yti, QA[:, nr * M : 2 * nr * M],
                            QB[:, : nr * M], -1, -1, nr * M)

            def fwd_gen(st):
                pivot_factors(st, 0)
                yield
                for k in range(A - 1):
                    update_plane(st, k, k + 1, critical=True)
                    yield
                    pivot_factors(st, k + 1)
                    yield
                    for j in range(k + 2, NP):
                        update_plane(st, k, j)
                        yield

            def back_gen(st):
                ci = st["ci"]
                for k in range(A - 1, -1, -1):
                    X2 = solve_x(st, k)
                    yield
                    if k > 0:
                        clear_rows(st, k, X2, k - 1, 1, critical=True)
                        yield
                    nc.sync.dma_start(out[ci, k], X2)
                    yield
                    if k > 1:
                        clear_rows(st, k, X2, 0, k - 1, critical=False)
                        yield

            _drive(fwd_gen(states[0]))
            _drive(back_gen(states[0]), fwd_gen(states[1]))
            _drive(back_gen(states[1]))

    nc.finalize()
    if os.environ.get("BD_DEBUG"):
        print("balancer busy (ns):", {k: round(v) for k, v in bal.busy.items()})
    return nc


_NC_CACHE = None


def _get_nc():
    global _NC_CACHE
    if _NC_CACHE is None:
        _NC_CACHE = _build()
    return _NC_CACHE


def _prep_core(y_re, y_im, h_re, h_im, c):
    """Host-side shard prep for core c: f-slice, block-diagonal extraction,
    and pre-transposition into the on-chip compute layout."""
    fsl = slice(c * FS, (c + 1) * FS)
    ue = np.arange(U)
    maps = {}
    for name, h, y in (("hy_re", h_re, y_re), ("hy_im", h_im, y_im)):
        h6 = h[:, 0, :, :, :, :, fsl].reshape(B, U, A, U, A, S, FS)
        hd = h6[:, ue, :, ue]                    # [u, b, i, j, s, f]
        hdt = hd.transpose(5, 3, 2, 0, 1, 4)     # [f, j, i, u, b, s]
        y5 = y[:, 0, :, :, fsl].reshape(B, U, A, S, FS)   # [b, u, i, s, f]
        yt = y5.transpose(4, 2, 1, 0, 3)         # [f, i, u, b, s]
        sup = np.empty((NCH, FS, NP, A, U, NB, S), np.float32)
        for ci in range(NCH):
            bsl = slice(ci * NB, (ci + 1) * NB)
            sup[ci, :, :A] = hdt[:, :, :, :, bsl]
            sup[ci, :, A] = yt[:, :, :, bsl]
        maps[name] = np.ascontiguousarray(sup.reshape(NCH, FS, NP * A * M))
    return maps


def kernel(y_re, y_im, h_re, h_im, **_ignored):
    global LAST_RESULTS
    y_re = np.asarray(y_re, dtype=np.float32)
    y_im = np.asarray(y_im, dtype=np.float32)
    h_re = np.asarray(h_re, dtype=np.float32)
    h_im = np.asarray(h_im, dtype=np.float32)

    nc = _get_nc()
    in_maps = [_prep_core(y_re, y_im, h_re, h_im, c) for c in range(NCORES)]
    trace = bool(int(os.environ.get("BD_TRACE", "0")))
    res = run_bass_kernel_spmd(
        nc, in_maps, core_ids=list(range(NCORES)), trace=trace
    )
    LAST_RESULTS = res
    outs = []
    for r in res.results:
        o = r["out"]                              # [ci, k, f, (c, u, b', s)]
        o = o.reshape(NCH, A, FS, 2, U, NB, S)
        o = o.transpose(0, 5, 4, 1, 6, 2, 3)      # [ci, b', u, k, s, f, c]
        o = o.reshape(B, U * A, S, FS, 2)         # [b, (u,i)=nr, s, f, c]
        outs.append(o)
    full = np.concatenate(outs, axis=3)           # [B, NR, S, F, 2]
    return np.ascontiguousarray(full[:, None])    # [B, 1, NR, S, F, 2]
